# revision 25
# baseline (speedup 1.0000x reference)
"""BinConv3d (sign-binarized 3x3x3 conv, NCDHW) on 8 Trainium2 NeuronCores.

Full inputs in, full output out. Sharding: depth dim D=64 split 8 ways
(8 output planes per core) with a 1-plane halo on the input; conv weights
and bias replicated to every core.

Host prep: each core's input slab is rearranged to [plane, slab, 128,
10, 130] fp32 — H split into 4 quarter-row panels (partition = q*32+ci),
each panel's padded 34 rows stored as 4 overlapping 10-row slabs
([0,10) [8,18) [16,26) [24,34)), so every DMA is one fully contiguous
5200B run per partition and slab s exactly covers the rows needed by
output row-block pair s.

Per-core kernel (Bass/Tile):
  - One [128, 10*130] DMA per (plane, slab), then ScalarE Sign
    fp32 -> bf16 (zero pads stay zero).
  - Conv = 27 accumulating matmuls (K=32 ci, M=64 co, N=512) per 4-row
    output tile; every (kd, kh, kw) tap is a free-dim AP offset.
  - 16-way PE tiling: quarter q runs on PE row-group q (tile_position
    row 32q); even/odd 4-row blocks run on PE column halves. 8 matmuls
    issue back-to-back per tap and run concurrently: full 128x128 array.
  - PSUM: 4 banks per generation (bank = quarter, partitions 0-63 even
    block / 64-127 odd block), double-buffered = all 8 banks.
  - PSUM drained with bias add on ScalarE (even) / VectorE (odd) into a
    [128, 2048] fp16 staging tile, flushed to HBM as one 512KB DMA with
    4KB-contiguous per-partition runs; the host transposes the staged
    [d, pi, half*64+co, q*512+r*128+w] layout back to NCDHW fp32.
"""

import numpy as np
import ml_dtypes

import concourse.bass as bass
import concourse.mybir as mybir
import concourse.tile as tile
from concourse import bacc
from concourse.bass import ts
from concourse.bass_utils import run_bass_kernel_spmd
from concourse.tile_rust import add_dep_helper

CI = 32
CO = 64
D_FULL = 64
N_CORES = 8
D_OUT = D_FULL // N_CORES  # output planes per core
D_IN = D_OUT + 2  # input planes per core (1-plane halo each side)
SLAB = 10  # rows per input slab (8 + 2 halo)
WP = 130  # padded width

_cache = {}


def build_conv_program(n_in_planes=D_IN, n_out_planes=D_OUT, H=128, W=128,
                       debug=False):
    """Build the per-core Bass program (SPMD: same program on all cores)."""
    f32 = mybir.dt.float32
    f16 = mybir.dt.float16
    bf16 = mybir.dt.bfloat16
    n_slabs = H // 32  # 4: one slab per output row-block pair per quarter
    assert H == 128 and W == 128

    nc = bacc.Bacc("TRN2", target_bir_lowering=False, debug=debug)
    x_in = nc.declare_dram_parameter(
        "xs", [n_in_planes, n_slabs, 128, SLAB, WP], f32, isOutput=False)
    w_in = nc.declare_dram_parameter("wst", [128, 27, 2 * CO], bf16,
                                     isOutput=False)
    b_in = nc.declare_dram_parameter("bias", [128, 1], f32, isOutput=False)
    y_out = nc.declare_dram_parameter(
        "y", [n_out_planes, n_slabs, 128, 4 * 512], f16, isOutput=True)

    with tile.TileContext(nc) as tc:
        with (
            tc.tile_pool(name="const", bufs=1) as constp,
            tc.tile_pool(name="raw", bufs=8) as rawp,
            tc.tile_pool(name="sgn", bufs=16) as sgnp,
            tc.tile_pool(name="stg", bufs=8) as stgp,
            tc.tile_pool(name="psum", bufs=2, space="PSUM") as psump,
        ):
            wt = constp.tile([128, 27, 2 * CO], bf16)
            nc.sync.dma_start(out=wt[:], in_=w_in[:])
            bs = constp.tile([128, 1], f32)
            nc.sync.dma_start(out=bs[:], in_=b_in[:])

            sgns = {}
            # the scheduler reorders per-engine queues and will put later
            # DMA-gated signs ahead of ready drains, blocking PSUM
            # recycling -- chain every ScalarE op to pin program order
            scal_chain = [None]

            def on_scalar(ins):
                if scal_chain[0] is not None:
                    add_dep_helper(ins.ins, scal_chain[0].ins,
                                   reason="scalar program order")
                scal_chain[0] = ins

            def load_slab(p, s):
                raw = rawp.tile([128, SLAB, WP], f32, tag="raw")
                nc.sync.dma_start(out=raw[:], in_=x_in[p, s])
                sgn = sgnp.tile([128, SLAB, WP], bf16, tag="sgn")
                on_scalar(nc.scalar.sign(sgn[:], raw[:]))
                sgns[(p, s)] = sgn

            # pi-major gen order: each slab column pi consumes one fresh
            # slab per gen (vs 3/gen for d-major during d=0), matching the
            # serial ~2.2us/slab input queue rate with no bursts
            for p in range(4):
                load_slab(p, 0)

            for pi in range(n_slabs):
                for d in range(n_out_planes):
                    # generation: for each quarter, blocks 2*pi (even,
                    # cols 0-63) and 2*pi+1 (odd, cols 64-127)
                    pts = [psump.tile([128, 512], f32, tag=f"pt{q}",
                                      name=f"pt{q}_{d}_{pi}")
                           for q in range(4)]
                    for tap in range(27):
                        kd, r = divmod(tap, 9)
                        kh, kw = divmod(r, 3)
                        sg = sgns[(d + kd, pi)]
                        for q in range(4):
                            for half in range(2):
                                rhs = sg[32 * q:32 * q + 32,
                                         4 * half + kh:4 * half + kh + 4,
                                         kw:kw + W]
                                nc.tensor.matmul(
                                    pts[q][64 * half:64 * half + 64, :],
                                    lhsT=wt[32 * q:32 * q + 32, tap, 0:CO],
                                    rhs=rhs,
                                    start=(tap == 0),
                                    stop=(tap == 26),
                                    tile_position=(32 * q, 64 * half),
                                    skip_group_check=True,
                                )
                    # drain with bias add, split 3:5 between ScalarE and
                    # VectorE (ScalarE also runs the signs); the last gen
                    # splits 4:4 (no signs left) to shorten the tail
                    last = (d == n_out_planes - 1 and pi == n_slabs - 1)
                    n_scal = 4 if last else 3
                    stg = stgp.tile([128, 4 * 512], f16, tag="stg")
                    for q in range(4):
                        if q < n_scal:
                            on_scalar(nc.scalar.activation(
                                stg[0:64, ts(q, 512)], pts[q][0:64, :],
                                mybir.ActivationFunctionType.Identity,
                                bias=bs[0:64], scale=1.0,
                            ))
                        else:
                            nc.vector.tensor_scalar_add(
                                out=stg[0:64, ts(q, 512)], in0=pts[q][0:64, :],
                                scalar1=bs[0:64],
                            )
                        nc.vector.tensor_scalar_add(
                            out=stg[64:128, ts(q, 512)], in0=pts[q][64:128, :],
                            scalar1=bs[64:128],
                        )
                    if last:
                        # two partition-half DMAs so the store can start as
                        # soon as the scalar half of the drain finishes
                        nc.sync.dma_start(out=y_out[d, pi, 0:64],
                                          in_=stg[0:64, :])
                        nc.sync.dma_start(out=y_out[d, pi, 64:128],
                                          in_=stg[64:128, :])
                    else:
                        nc.sync.dma_start(out=y_out[d, pi], in_=stg[:])
                    # prefetch: continue down this slab column, then start
                    # the next column during the last two gens
                    if d + 4 < n_in_planes:
                        load_slab(d + 4, pi)
                    elif pi + 1 < n_slabs:
                        base = 2 * (d - (n_out_planes - 2))
                        load_slab(base, pi + 1)
                        load_slab(base + 1, pi + 1)

    nc.compile()
    return nc


def _get_program():
    if "nc" not in _cache:
        _cache["nc"] = build_conv_program()
    return _cache["nc"]


def prep_weights(W, b):
    W = np.asarray(W, dtype=np.float32)
    b = np.asarray(b, dtype=np.float32)
    # wst[q*32+ci, kd*9+kh*3+kw, half*64+co] = W[co, ci, kd, kh, kw],
    # replicated over the 4 row groups and the 2 column halves
    wq = W.transpose(1, 2, 3, 4, 0).reshape(CI, 27, CO)
    wq2 = np.concatenate([wq, wq], axis=2)
    wst = np.ascontiguousarray(
        np.broadcast_to(wq2[None], (4, CI, 27, 2 * CO)).reshape(128, 27, 2 * CO)
    ).astype(ml_dtypes.bfloat16)
    bias = np.ascontiguousarray(
        np.concatenate([b, b]).reshape(128, 1).astype(np.float32))
    return wst, bias


def prep_x_slab(xpad, p_lo, n_planes, H=128):
    """xpad: [CI, D+2, H+2, W+2] zero-padded input. Returns
    [n_planes, 4, 128, SLAB, WP] fp32 for planes p_lo..p_lo+n_planes;
    partition dim = q*32+ci, slab s = padded quarter rows [8s, 8s+10)."""
    out = np.empty((n_planes, 4, 4, CI, SLAB, WP), dtype=np.float32)
    for q in range(4):
        for s in range(4):
            r0 = 32 * q + 8 * s
            out[:, s, q] = xpad[:, p_lo:p_lo + n_planes,
                                r0:r0 + SLAB, :].transpose(1, 0, 2, 3)
    return out.reshape(n_planes, 4, 128, SLAB, WP)


def _prep_inputs(x, W, b):
    x = np.asarray(x, dtype=np.float32)
    wst, bias = prep_weights(W, b)
    xpad = np.pad(x[0], ((0, 0), (1, 1), (1, 1), (1, 1)))
    in_maps = []
    for k in range(N_CORES):
        xs = prep_x_slab(xpad, D_OUT * k, D_IN)
        in_maps.append({"xs": xs, "wst": wst, "bias": bias})
    return in_maps


def _gather_output(res):
    # y_dev[k]: [D_OUT, 4, 128, 2048] fp16 with
    # [d, pi, half*64+co, q*512 + r*128 + w]
    parts = []
    for k in range(N_CORES):
        v = np.asarray(res.results[k]["y"])
        v = v.reshape(D_OUT, 4, 2, CO, 4, 4, 128)  # d pi half co q r w
        # out rows: 32q + 8pi + 4half + r
        v = v.transpose(3, 0, 4, 1, 2, 5, 6)  # co d q pi half r w
        parts.append(v.reshape(CO, D_OUT, 128, 128))
    y = np.concatenate(parts, axis=1).astype(np.float32)
    return y[None]


def run(x, W, b, trace=False):
    """Run the kernel; returns (output, BassKernelResults)."""
    nc = _get_program()
    in_maps = _prep_inputs(x, W, b)
    res = run_bass_kernel_spmd(nc, in_maps, list(range(N_CORES)), trace=trace)
    return _gather_output(res), res


def kernel(x, W, b):
    y, _ = run(x, W, b)
    return y


# revision 26
# speedup vs baseline: 1.0309x; 1.0309x over previous
"""BinConv3d (sign-binarized 3x3x3 conv, NCDHW) on 8 Trainium2 NeuronCores.

Full inputs in, full output out. Sharding: depth dim D=64 split 8 ways
(8 output planes per core) with a 1-plane halo on the input; conv weights
and bias replicated to every core.

Host prep: each core's input slab is rearranged to [plane, slab, 128,
10, 130] fp32 — H split into 4 quarter-row panels (partition = q*32+ci),
each panel's padded 34 rows stored as 4 overlapping 10-row slabs
([0,10) [8,18) [16,26) [24,34)), so every DMA is one fully contiguous
5200B run per partition and slab s exactly covers the rows needed by
output row-block pair s.

Per-core kernel (Bass/Tile):
  - One [128, 10*130] DMA per (plane, slab), then ScalarE Sign
    fp32 -> bf16 (zero pads stay zero).
  - Conv = 27 accumulating matmuls (K=32 ci, M=64 co, N=512) per 4-row
    output tile; every (kd, kh, kw) tap is a free-dim AP offset.
  - 16-way PE tiling: quarter q runs on PE row-group q (tile_position
    row 32q); even/odd 4-row blocks run on PE column halves. 8 matmuls
    issue back-to-back per tap and run concurrently: full 128x128 array.
  - PSUM: 4 banks per generation (bank = quarter, partitions 0-63 even
    block / 64-127 odd block), double-buffered = all 8 banks.
  - PSUM drained with bias add on ScalarE (even) / VectorE (odd) into a
    [128, 2048] fp16 staging tile, flushed to HBM as one 512KB DMA with
    4KB-contiguous per-partition runs; the host transposes the staged
    [d, pi, half*64+co, q*512+r*128+w] layout back to NCDHW fp32.
"""

import numpy as np
import ml_dtypes

import concourse.bass as bass
import concourse.mybir as mybir
import concourse.tile as tile
from concourse import bacc
from concourse.bass import ts
from concourse.bass_utils import run_bass_kernel_spmd
from concourse.tile_rust import add_dep_helper

CI = 32
CO = 64
D_FULL = 64
N_CORES = 8
D_OUT = D_FULL // N_CORES  # output planes per core
D_IN = D_OUT + 2  # input planes per core (1-plane halo each side)
SLAB = 10  # rows per input slab (8 + 2 halo)
WP = 130  # padded width

_cache = {}


def build_conv_program(n_in_planes=D_IN, n_out_planes=D_OUT, H=128, W=128,
                       debug=False):
    """Build the per-core Bass program (SPMD: same program on all cores)."""
    f32 = mybir.dt.float32
    f16 = mybir.dt.float16
    bf16 = mybir.dt.bfloat16
    n_slabs = H // 32  # 4: one slab per output row-block pair per quarter
    assert H == 128 and W == 128

    nc = bacc.Bacc("TRN2", target_bir_lowering=False, debug=debug)
    x_in = nc.declare_dram_parameter(
        "xs", [n_in_planes, n_slabs, 128, SLAB, WP], f32, isOutput=False)
    w_in = nc.declare_dram_parameter("wst", [128, 27, 2 * CO], bf16,
                                     isOutput=False)
    b_in = nc.declare_dram_parameter("bias", [128, 1], f32, isOutput=False)
    y_out = nc.declare_dram_parameter(
        "y", [n_out_planes, n_slabs, 128, 4 * 512], f16, isOutput=True)

    with tile.TileContext(nc) as tc:
        with (
            tc.tile_pool(name="const", bufs=1) as constp,
            tc.tile_pool(name="raw", bufs=8) as rawp,
            tc.tile_pool(name="sgn", bufs=16) as sgnp,
            tc.tile_pool(name="stg", bufs=8) as stgp,
            tc.tile_pool(name="psum", bufs=2, space="PSUM") as psump,
        ):
            wt = constp.tile([128, 27, 2 * CO], bf16)
            nc.sync.dma_start(out=wt[:], in_=w_in[:])
            bs = constp.tile([128, 1], f32)
            nc.sync.dma_start(out=bs[:], in_=b_in[:])

            sgns = {}
            # the scheduler reorders per-engine queues and will put later
            # DMA-gated signs ahead of ready drains, blocking PSUM
            # recycling -- chain every ScalarE op to pin program order
            scal_chain = [None]

            def on_scalar(ins):
                if scal_chain[0] is not None:
                    add_dep_helper(ins.ins, scal_chain[0].ins,
                                   reason="scalar program order")
                scal_chain[0] = ins

            def load_slab(p, s):
                raw = rawp.tile([128, SLAB, WP], f32, tag="raw")
                nc.sync.dma_start(out=raw[:], in_=x_in[p, s])
                sgn = sgnp.tile([128, SLAB, WP], bf16, tag="sgn")
                on_scalar(nc.scalar.sign(sgn[:], raw[:]))
                sgns[(p, s)] = sgn

            # pi-major gen order: each slab column pi consumes one fresh
            # slab per gen (vs 3/gen for d-major during d=0), matching the
            # serial ~2.2us/slab input queue rate with no bursts
            for p in range(4):
                load_slab(p, 0)

            for pi in range(n_slabs):
                for d in range(n_out_planes):
                    # generation: for each quarter, blocks 2*pi (even,
                    # cols 0-63) and 2*pi+1 (odd, cols 64-127)
                    pts = [psump.tile([128, 512], f32, tag=f"pt{q}",
                                      name=f"pt{q}_{d}_{pi}")
                           for q in range(4)]
                    for tap in range(27):
                        kd, r = divmod(tap, 9)
                        kh, kw = divmod(r, 3)
                        sg = sgns[(d + kd, pi)]
                        for q in range(4):
                            for half in range(2):
                                rhs = sg[32 * q:32 * q + 32,
                                         4 * half + kh:4 * half + kh + 4,
                                         kw:kw + W]
                                nc.tensor.matmul(
                                    pts[q][64 * half:64 * half + 64, :],
                                    lhsT=wt[32 * q:32 * q + 32, tap, 0:CO],
                                    rhs=rhs,
                                    start=(tap == 0),
                                    stop=(tap == 26),
                                    tile_position=(32 * q, 64 * half),
                                    skip_group_check=True,
                                )
                    # drain with bias add. All drains go to VectorE (they
                    # fit in the gen period and this fully decouples PSUM
                    # recycling from the DMA-gated signs on ScalarE); the
                    # last gen splits 4:4 to shorten the tail.
                    last = (d == n_out_planes - 1 and pi == n_slabs - 1)
                    n_scal = 4 if last else 0
                    stg = stgp.tile([128, 4 * 512], f16, tag="stg")
                    for q in range(4):
                        if q < n_scal:
                            on_scalar(nc.scalar.activation(
                                stg[0:64, ts(q, 512)], pts[q][0:64, :],
                                mybir.ActivationFunctionType.Identity,
                                bias=bs[0:64], scale=1.0,
                            ))
                        else:
                            nc.vector.tensor_scalar_add(
                                out=stg[0:64, ts(q, 512)], in0=pts[q][0:64, :],
                                scalar1=bs[0:64],
                            )
                        nc.vector.tensor_scalar_add(
                            out=stg[64:128, ts(q, 512)], in0=pts[q][64:128, :],
                            scalar1=bs[64:128],
                        )
                    if last:
                        # two partition-half DMAs so the store can start as
                        # soon as the scalar half of the drain finishes
                        nc.sync.dma_start(out=y_out[d, pi, 0:64],
                                          in_=stg[0:64, :])
                        nc.sync.dma_start(out=y_out[d, pi, 64:128],
                                          in_=stg[64:128, :])
                    else:
                        nc.sync.dma_start(out=y_out[d, pi], in_=stg[:])
                    # prefetch: continue down this slab column, then start
                    # the next column during the last two gens
                    if d + 4 < n_in_planes:
                        load_slab(d + 4, pi)
                    elif pi + 1 < n_slabs:
                        base = 2 * (d - (n_out_planes - 2))
                        load_slab(base, pi + 1)
                        load_slab(base + 1, pi + 1)

    nc.compile()
    return nc


def _get_program():
    if "nc" not in _cache:
        _cache["nc"] = build_conv_program()
    return _cache["nc"]


def prep_weights(W, b):
    W = np.asarray(W, dtype=np.float32)
    b = np.asarray(b, dtype=np.float32)
    # wst[q*32+ci, kd*9+kh*3+kw, half*64+co] = W[co, ci, kd, kh, kw],
    # replicated over the 4 row groups and the 2 column halves
    wq = W.transpose(1, 2, 3, 4, 0).reshape(CI, 27, CO)
    wq2 = np.concatenate([wq, wq], axis=2)
    wst = np.ascontiguousarray(
        np.broadcast_to(wq2[None], (4, CI, 27, 2 * CO)).reshape(128, 27, 2 * CO)
    ).astype(ml_dtypes.bfloat16)
    bias = np.ascontiguousarray(
        np.concatenate([b, b]).reshape(128, 1).astype(np.float32))
    return wst, bias


def prep_x_slab(xpad, p_lo, n_planes, H=128):
    """xpad: [CI, D+2, H+2, W+2] zero-padded input. Returns
    [n_planes, 4, 128, SLAB, WP] fp32 for planes p_lo..p_lo+n_planes;
    partition dim = q*32+ci, slab s = padded quarter rows [8s, 8s+10)."""
    out = np.empty((n_planes, 4, 4, CI, SLAB, WP), dtype=np.float32)
    for q in range(4):
        for s in range(4):
            r0 = 32 * q + 8 * s
            out[:, s, q] = xpad[:, p_lo:p_lo + n_planes,
                                r0:r0 + SLAB, :].transpose(1, 0, 2, 3)
    return out.reshape(n_planes, 4, 128, SLAB, WP)


def _prep_inputs(x, W, b):
    x = np.asarray(x, dtype=np.float32)
    wst, bias = prep_weights(W, b)
    xpad = np.pad(x[0], ((0, 0), (1, 1), (1, 1), (1, 1)))
    in_maps = []
    for k in range(N_CORES):
        xs = prep_x_slab(xpad, D_OUT * k, D_IN)
        in_maps.append({"xs": xs, "wst": wst, "bias": bias})
    return in_maps


def _gather_output(res):
    # y_dev[k]: [D_OUT, 4, 128, 2048] fp16 with
    # [d, pi, half*64+co, q*512 + r*128 + w]
    parts = []
    for k in range(N_CORES):
        v = np.asarray(res.results[k]["y"])
        v = v.reshape(D_OUT, 4, 2, CO, 4, 4, 128)  # d pi half co q r w
        # out rows: 32q + 8pi + 4half + r
        v = v.transpose(3, 0, 4, 1, 2, 5, 6)  # co d q pi half r w
        parts.append(v.reshape(CO, D_OUT, 128, 128))
    y = np.concatenate(parts, axis=1).astype(np.float32)
    return y[None]


def run(x, W, b, trace=False):
    """Run the kernel; returns (output, BassKernelResults)."""
    nc = _get_program()
    in_maps = _prep_inputs(x, W, b)
    res = run_bass_kernel_spmd(nc, in_maps, list(range(N_CORES)), trace=trace)
    return _gather_output(res), res


def kernel(x, W, b):
    y, _ = run(x, W, b)
    return y
